# revision 6
# baseline (speedup 1.0000x reference)
"""Trainium2 Bass kernel for nn_Compressor: audio dynamic-range compressor.

Reference computation (per sample, sequential over the audio axis):
    db   = 20*log10(|a| + 1e-5)
    grd  = max((threshold - db) * (1 - 1/ratio), 0)
    g_t  = g_{t-1} + (1 - c_t) * (grd_t - g_{t-1}),  c_t = attack if grd_t > g_{t-1} else release
    out  = a * 10^(-g/20)

Key identities used here:
  * The data-dependent branch collapses into a max:
        g_t = g_{t-1} + max(ma*d, mr*d),   d = grd_t - g_{t-1},
        ma = 1-attack, mr = 1-release  (ma > mr)
    (when rising, the attack branch is the larger candidate; when falling, the
    release branch is -- bit-exact vs. the reference branch arithmetic).
  * The step map g -> g + max(ma*(x-g), mr*(x-g)) is a contraction with
    Lipschitz constant max(attack, release) = 0.1, so a chunk of the signal
    computed from an arbitrary initial state converges to the exact
    trajectory after W warmup samples (error <= 60 * 0.1^W).

Parallelization: split the stream into chunks of T samples with W-sample
warmup halos; all chunks evolve in lockstep on the 128 SBUF partitions via a
custom DVE op, with a time-major [t, chunk] SBUF layout produced for free by
transposing access patterns on the ScalarEngine activation passes.

8 NeuronCores each take 1/8 of the stream (plus a W-sample halo).
"""

import os
import numpy as np

NCORES = 8
ROWS = 128

# Tiling configuration (overridable for experiments via env)
W = int(os.environ.get("COMP_W", 8))        # warmup samples per chunk
T = int(os.environ.get("COMP_T", 32))       # kept samples per chunk
C = int(os.environ.get("COMP_C", 128))      # chunks per partition row
SCAN_MODE = os.environ.get("COMP_SCAN", "steps")  # "steps" | "fused"
ABS_ENGINE = os.environ.get("COMP_ABS", "act")    # "act" | "dve"

TP = T + W
CT = C * T
TILE = ROWS * CT

_OP_CACHE = {}
_PROG_CACHE = {}


def _register_scan_op():
    """Register the custom DVE op  out = in1 + max((in0-in1)*s0, (in0-in1)*s1)
    (one compressor smoothing step for a [P, C] slab of chunks)."""
    if "op" in _OP_CACHE:
        return _OP_CACHE["op"]
    import concourse.dve_ops as dvo
    from concourse.dve_spec import Spec, Src0, Src1, C0, C1, maxx, lower
    from concourse.dve_uop import DveOpSpec

    name = "COMPRESSOR_SCAN_STEP_ANT"
    d = Src0 - Src1
    spec = Spec(
        body=Src1 + maxx(d * C0, d * C1),
        reference=lambda in0, in1, c0, c1, c2: (
            in1
            + np.maximum(
                ((in0 - in1) * np.float32(c0)).astype(np.float32),
                ((in0 - in1) * np.float32(c1)).astype(np.float32),
            )
        ).astype(np.float32),
    )
    if name not in dvo._SUB_OPCODE_FOR_NAME:
        row = dvo._CUSTOM_DVE_ROW_BASE + len(dvo.OPS)
        dvo._SUB_OPCODE_FOR_NAME[name] = row
        shas = {}
        for ver in ("v3", "v4"):
            uops = lower(spec, ver=ver)
            shas[ver] = DveOpSpec(
                name=name, opcode=row, uops=uops, rd1_en=True
            ).sha(ver)
        op = dvo.DveOp(name, spec, subdim=False, uops_sha=shas)
        dvo.OPS.append(op)
        dvo.CUSTOM_DVE_SPECS[name] = spec
    else:
        op = next(o for o in dvo.OPS if o.name == name)
    _OP_CACHE["op"] = op
    return op


def _fv(tile_ap, off, dims):
    """Free-dim view of a 2D/3D SBUF tile AP: keep the partition dim, replace
    the free dims with explicit (stride, num) pairs; offset in elements."""
    from concourse.ap import AP

    pairs = [list(tile_ap.ap[0])] + [[int(s), int(n)] for s, n in dims]
    return AP(tile_ap.tensor, tile_ap.offset + int(off), pairs)


def _build_program(L, threshold, ratio, attack, release):
    from concourse import bacc, tile, mybir

    op = _register_scan_op()
    f32 = mybir.dt.float32
    ACT = mybir.ActivationFunctionType

    ntiles = L // TILE
    assert L % TILE == 0, (L, TILE)

    # constants (f32 arithmetic mirroring the reference)
    ma = float(np.float32(1.0) - np.float32(attack))    # bigger multiplier
    mr = float(np.float32(1.0) - np.float32(release))
    q = float(np.float32(1.0) - np.float32(1.0) / np.float32(ratio))
    grd_scale = -q * 20.0 / float(np.log(10.0))         # grd = relu(gs*ln(u) + gb)
    grd_bias = float(np.float32(threshold)) * q
    gain_scale = -float(np.log(10.0)) / 20.0            # gain = exp(gscale*g)

    nc = bacc.Bacc(
        "TRN2", target_bir_lowering=False, debug=False, num_devices=NCORES
    )

    # activation biases must exist as const APs
    for value in (1e-5, grd_bias):
        if (f32, value) not in nc.const_aps.aps:
            ct = nc.alloc_sbuf_tensor(f"const-f32-{value}", [128, 1], f32)
            nc.gpsimd.memset(ct.ap(), value)
            nc.const_aps.aps[(f32, value)] = ct.ap()
    nc.all_engine_barrier()

    audio_in = nc.dram_tensor(
        "audio", [L + W], f32, kind="ExternalInput"
    ).ap()
    out_t = nc.dram_tensor("out", [L], f32, kind="ExternalOutput").ap()

    with tile.TileContext(nc) as tc:
        with tc.tile_pool(name="apool", bufs=2) as apool, \
             tc.tile_pool(name="bpool", bufs=2) as bpool, \
             tc.tile_pool(name="gpool", bufs=2) as gpool:
            for j in range(ntiles):
                base = j * TILE
                A = apool.tile([ROWS, W + CT], f32, name="A")
                # main body: contiguous [ROWS, CT]
                nc.sync.dma_start(
                    A[:, W:],
                    audio_in[W + base : W + base + TILE].rearrange(
                        "(p f) -> p f", p=ROWS
                    ),
                )
                # per-row halo: W samples preceding each row
                nc.sync.dma_start(
                    A[:, 0:W],
                    audio_in[base : base + TILE].rearrange(
                        "(p f) -> p f", p=ROWS
                    )[:, 0:W],
                )

                B = bpool.tile([ROWS, W + CT], f32, name="B")
                if ABS_ENGINE == "dve":
                    u32 = mybir.dt.uint32
                    nc.vector.tensor_scalar(
                        B[:].bitcast(u32),
                        A[:].bitcast(u32),
                        0x7FFFFFFF,
                        None,
                        mybir.AluOpType.bitwise_and,
                    )
                else:
                    nc.scalar.activation(B[:], A[:], ACT.Abs)
                nc.scalar.activation(B[:], B[:], ACT.Ln, bias=1e-5)

                # grd with a transposing write into time-major layout:
                #   G[p, t, c] = relu(grd_scale * lnval[p, c*T + t] + grd_bias)
                G = gpool.tile([ROWS, TP, C], f32, name="G")
                gin = _fv(B, 0, [(1, TP), (T, C)])
                nc.scalar.activation(
                    G[:, :, :], gin, ACT.Relu, bias=grd_bias, scale=grd_scale
                )

                # the smoothing scan: each step updates all ROWS*C chunks
                if SCAN_MODE == "fused":
                    nc.vector._custom_dve(
                        op,
                        out=G[:, 1:TP, :],
                        in0=G[:, 1:TP, :],
                        in1=G[:, 0 : TP - 1, :],
                        s0=ma,
                        s1=mr,
                    )
                else:
                    for t in range(1, TP):
                        nc.vector._custom_dve(
                            op,
                            out=G[:, t, :],
                            in0=G[:, t, :],
                            in1=G[:, t - 1, :],
                            s0=ma,
                            s1=mr,
                        )

                # gain with a transposing read back to natural layout:
                #   B[p, c*T + tau] = exp(gain_scale * G[p, W + tau, c])
                ggain_in = _fv(G, W * C, [(1, C), (C, T)])
                ggain_out = _fv(B, 0, [(T, C), (1, T)])
                nc.scalar.activation(ggain_out, ggain_in, ACT.Exp, scale=gain_scale)

                # out = audio * gain  (natural layout, in place over the gain)
                nc.vector.tensor_tensor(
                    B[:, 0:CT], A[:, W:], B[:, 0:CT], mybir.AluOpType.mult
                )

                nc.sync.dma_start(
                    out_t[base : base + TILE].rearrange("(p f) -> p f", p=ROWS),
                    B[:, 0:CT],
                )
    nc.compile()
    return nc


def _get_program(L, threshold, ratio, attack, release):
    key = (L, threshold, ratio, attack, release, W, T, C, SCAN_MODE, ABS_ENGINE)
    if key not in _PROG_CACHE:
        _PROG_CACHE[key] = _build_program(L, threshold, ratio, attack, release)
    return _PROG_CACHE[key]


def _ensure_ntff_hook():
    """The image's `antenv` lacks `axon_hooks`; synthesize it so
    run_bass_kernel_spmd(trace=True) can reach the ctypes NTFF profiler.
    Also stub out the S3 artifact upload (no creds in this sandbox)."""
    import sys
    import types

    if "antenv.axon_hooks" not in sys.modules:
        m = types.ModuleType("antenv.axon_hooks")
        _hook = [None]

        def set_axon_ntff_profile_hook(h):
            _hook[0] = h

        def get_axon_ntff_profile_hook():
            if _hook[0] is None:
                try:
                    from trn_agent_boot.trn_boot import _ntff_profile_via_ctypes

                    _hook[0] = _ntff_profile_via_ctypes("/opt/axon/libaxon_pjrt.so")
                except Exception:
                    return None
            return _hook[0]

        m.set_axon_ntff_profile_hook = set_axon_ntff_profile_hook
        m.get_axon_ntff_profile_hook = get_axon_ntff_profile_hook
        sys.modules["antenv.axon_hooks"] = m

    import concourse.bass_utils as bu

    if not getattr(bu.upload_artifacts, "_comp_stubbed", False):
        def _no_upload(tmpdir):
            return f"file://{tmpdir}"

        _no_upload._comp_stubbed = True
        bu.upload_artifacts = _no_upload


def kernel(
    audio=None,
    sample_rate=None,
    threshold=None,
    ratio=None,
    attack=None,
    release=None,
    **_unused,
):
    audio_np = np.asarray(audio, dtype=np.float32).reshape(-1)
    n = audio_np.size
    assert n % NCORES == 0, n
    L = n // NCORES

    th = float(np.float32(threshold))
    rt = float(np.float32(ratio))
    at = float(np.float32(attack))
    rl = float(np.float32(release))

    nc = _get_program(L, th, rt, at, rl)

    in_maps = []
    for k in range(NCORES):
        lo = k * L
        if k == 0:
            # grd(1.0) == 0 for any sane threshold < 0, and the scan state
            # initialized from it is exactly the reference's g0 = 0.
            halo = np.full(W, 1.0, dtype=np.float32)
        else:
            halo = audio_np[lo - W : lo]
        in_maps.append(
            {"audio": np.ascontiguousarray(np.concatenate([halo, audio_np[lo : lo + L]]))}
        )

    from concourse.bass_utils import run_bass_kernel_spmd

    trace = bool(int(os.environ.get("COMP_TRACE", "0")))
    if trace:
        _ensure_ntff_hook()
    res = run_bass_kernel_spmd(
        nc, in_maps, core_ids=list(range(NCORES)), trace=trace
    )
    if trace:
        print(f"HW exec time: {res.exec_time_ns} ns")
        print(f"mean exec time: {res.mean_exec_time_ns} ns")
    out = np.concatenate([res.results[k]["out"] for k in range(NCORES)])
    return out


# revision 14
# speedup vs baseline: 1.3167x; 1.3167x over previous
"""Trainium2 Bass kernel for nn_Compressor: audio dynamic-range compressor.

Reference computation (per sample, sequential over the audio axis):
    db   = 20*log10(|a| + 1e-5)
    grd  = max((threshold - db) * (1 - 1/ratio), 0)
    g_t  = g_{t-1} + (1 - c_t) * (grd_t - g_{t-1}),  c_t = attack if grd_t > g_{t-1} else release
    out  = a * 10^(-g/20)

Key identities used here:
  * The data-dependent branch collapses into a max:
        g_t = g_{t-1} + max(ma*d, mr*d),   d = grd_t - g_{t-1},
        ma = 1-attack, mr = 1-release  (ma > mr)
    (when rising, the attack branch is the larger candidate; when falling, the
    release branch is -- bit-exact vs. the reference branch arithmetic).
  * The step map g -> g + max(ma*(x-g), mr*(x-g)) is a contraction with
    Lipschitz constant max(attack, release) = 0.1, so a chunk of the signal
    computed from an arbitrary initial state converges to the exact
    trajectory after W warmup samples (error <= 60 * 0.1^W).

Parallelization: split the stream into chunks of T samples with W-sample
warmup halos; all chunks evolve in lockstep on the 128 SBUF partitions via a
custom DVE op, with a time-major [t, chunk] SBUF layout produced for free by
transposing access patterns on the ScalarEngine activation passes.

8 NeuronCores each take 1/8 of the stream (plus a W-sample halo).
"""

import os
import numpy as np

NCORES = 8
ROWS = 128

# Tiling configuration (overridable for experiments via env)
W = int(os.environ.get("COMP_W", 8))        # warmup samples per chunk
T = int(os.environ.get("COMP_T", 32))       # kept samples per chunk
C = int(os.environ.get("COMP_C", 256))      # chunks per partition row
ABS_ENGINE = os.environ.get("COMP_ABS", "act")    # "act" | "dve"
MULT_ENGINE = os.environ.get("COMP_MULT", "pool")  # "dve" | "pool"
GROUP = int(os.environ.get("COMP_GROUP", 2))       # interleaved scan chains

TP = T + W
CT = C * T
TILE = ROWS * CT

_OP_CACHE = {}
_PROG_CACHE = {}


def _register_scan_op():
    """Register the custom DVE op
        out = in1 + max((relu(in0)-in1)*s0, (relu(in0)-in1)*s1)
    (one compressor smoothing step for a [P, C] slab of chunks; the relu
    finishes the grd computation so the affine pass needn't apply it)."""
    if "op" in _OP_CACHE:
        return _OP_CACHE["op"]
    import concourse.dve_ops as dvo
    from concourse.dve_spec import Spec, Src0, Src1, C0, C1, maxx, relu, lower
    from concourse.dve_uop import DveOpSpec

    name = "COMPRESSOR_SCAN_STEP2_ANT"

    def _ref(in0, in1, c0, c1, c2):
        x = np.maximum(in0, 0).astype(np.float32)
        d = (x - in1).astype(np.float32)
        return (
            in1
            + np.maximum(
                (d * np.float32(c0)).astype(np.float32),
                (d * np.float32(c1)).astype(np.float32),
            )
        ).astype(np.float32)

    d = relu(Src0) - Src1
    spec = Spec(body=Src1 + maxx(d * C0, d * C1), reference=_ref)
    if name not in dvo._SUB_OPCODE_FOR_NAME:
        row = dvo._CUSTOM_DVE_ROW_BASE + len(dvo.OPS)
        dvo._SUB_OPCODE_FOR_NAME[name] = row
        shas = {}
        for ver in ("v3", "v4"):
            uops = lower(spec, ver=ver)
            shas[ver] = DveOpSpec(
                name=name, opcode=row, uops=uops, rd1_en=True
            ).sha(ver)
        op = dvo.DveOp(name, spec, subdim=False, uops_sha=shas)
        dvo.OPS.append(op)
        dvo.CUSTOM_DVE_SPECS[name] = spec
    else:
        op = next(o for o in dvo.OPS if o.name == name)
    _OP_CACHE["op"] = op
    return op


def _fv(tile_ap, off, dims):
    """Free-dim view of a 2D/3D SBUF tile AP: keep the partition dim, replace
    the free dims with explicit (stride, num) pairs; offset in elements."""
    from concourse.ap import AP

    pairs = [list(tile_ap.ap[0])] + [[int(s), int(n)] for s, n in dims]
    return AP(tile_ap.tensor, tile_ap.offset + int(off), pairs)


def _build_program(L, threshold, ratio, attack, release):
    from concourse import bacc, tile, tile_utils, mybir

    # the default cap (192KiB/partition) wastes the 208KiB usable on TRN2
    try:
        tile_utils.TileConfig.max_sbuf_usage = 206 * 1024
    except Exception:
        pass

    op = _register_scan_op()
    f32 = mybir.dt.float32
    u32 = mybir.dt.uint32
    ACT = mybir.ActivationFunctionType
    ALU = mybir.AluOpType

    ntiles = L // TILE
    assert L % TILE == 0, (L, TILE)

    # constants (f32 arithmetic mirroring the reference)
    ma = float(np.float32(1.0) - np.float32(attack))    # bigger multiplier
    mr = float(np.float32(1.0) - np.float32(release))
    q = float(np.float32(1.0) - np.float32(1.0) / np.float32(ratio))
    grd_scale = -q * 20.0 / float(np.log(10.0))         # grd = relu(gs*ln(u) + gb)
    grd_bias = float(np.float32(threshold)) * q
    gain_scale = -float(np.log(10.0)) / 20.0            # gain = exp(gscale*g)

    nc = bacc.Bacc(
        "TRN2", target_bir_lowering=False, debug=False, num_devices=NCORES
    )

    # activation biases must exist as const APs
    for value in (1e-5,):
        if (f32, value) not in nc.const_aps.aps:
            ct = nc.alloc_sbuf_tensor(f"const-f32-{value}", [128, 1], f32)
            nc.gpsimd.memset(ct.ap(), value)
            nc.const_aps.aps[(f32, value)] = ct.ap()
    nc.all_engine_barrier()

    audio_in = nc.dram_tensor(
        "audio", [L + W], f32, kind="ExternalInput"
    ).ap()
    out_t = nc.dram_tensor("out", [L], f32, kind="ExternalOutput").ap()

    GRP = min(GROUP, ntiles)
    assert ntiles % GRP == 0

    with tile.TileContext(nc) as tc:
        with tc.tile_pool(name="apool", bufs=2 * GRP) as apool, \
             tc.tile_pool(name="bpool", bufs=2 * GRP) as bpool, \
             tc.tile_pool(name="opool", bufs=GRP + 1) as opool, \
             tc.tile_pool(name="spool", bufs=2 * GRP) as spool:
            for g0 in range(0, ntiles, GRP):
                tiles = []
                for j in range(g0, g0 + GRP):
                    base = j * TILE
                    A = apool.tile([ROWS, W + CT], f32, name="A")
                    # main body: contiguous [ROWS, CT]
                    nc.sync.dma_start(
                        A[:, W:],
                        audio_in[W + base : W + base + TILE].rearrange(
                            "(p f) -> p f", p=ROWS
                        ),
                    )
                    # per-row halo: W samples preceding each row
                    nc.sync.dma_start(
                        A[:, 0:W],
                        audio_in[base : base + TILE].rearrange(
                            "(p f) -> p f", p=ROWS
                        )[:, 0:W],
                    )

                    # B = grd_scale*ln(|a|+1e-5) + grd_bias (relu in scan op)
                    B = bpool.tile([ROWS, W + CT], f32, name="B")
                    if ABS_ENGINE == "dve":
                        nc.vector.tensor_scalar(
                            B[:].bitcast(u32), A[:].bitcast(u32),
                            0x7FFFFFFF, None, ALU.bitwise_and,
                        )
                    else:
                        nc.scalar.activation(B[:], A[:], ACT.Abs)
                    nc.scalar.activation(B[:], B[:], ACT.Ln, bias=1e-5)
                    nc.vector.tensor_scalar(
                        B[:], B[:], grd_scale, grd_bias, ALU.mult, ALU.add
                    )
                    O = opool.tile([ROWS, CT], f32, name="O")
                    S = spool.tile([ROWS, C], f32, name="S")
                    nc.gpsimd.memset(S[:], 0.0)
                    tiles.append((base, A, B, O, S))

                # smoothing scan, GRP independent chains interleaved so the
                # DVE write->read turnaround bubble of one chain hides under
                # the other chains' execution.
                for t in range(TP):
                    for (base, A, B, O, S) in tiles:
                        x = _fv(B, t, [(T, C)])
                        prev = S[:] if t <= W else _fv(O, t - 1 - W, [(T, C)])
                        dst = S[:] if t < W else _fv(O, t - W, [(T, C)])
                        nc.vector._custom_dve(
                            op, out=dst, in0=x, in1=prev, s0=ma, s1=mr
                        )

                for (base, A, B, O, S) in tiles:
                    # gain = exp(gain_scale * g), then out = audio * gain
                    nc.scalar.activation(O[:], O[:], ACT.Exp, scale=gain_scale)
                    if MULT_ENGINE == "pool":
                        nc.gpsimd.tensor_tensor(O[:], A[:, W:], O[:], ALU.mult)
                    else:
                        nc.vector.tensor_tensor(O[:], A[:, W:], O[:], ALU.mult)
                    nc.sync.dma_start(
                        out_t[base : base + TILE].rearrange(
                            "(p f) -> p f", p=ROWS
                        ),
                        O[:],
                    )
    nc.compile()
    return nc


def _get_program(L, threshold, ratio, attack, release):
    key = (L, threshold, ratio, attack, release, W, T, C, ABS_ENGINE, MULT_ENGINE, GROUP)
    if key not in _PROG_CACHE:
        _PROG_CACHE[key] = _build_program(L, threshold, ratio, attack, release)
    return _PROG_CACHE[key]


def _ensure_ntff_hook():
    """The image's `antenv` lacks `axon_hooks`; synthesize it so
    run_bass_kernel_spmd(trace=True) can reach the ctypes NTFF profiler.
    Also stub out the S3 artifact upload (no creds in this sandbox)."""
    import sys
    import types

    if "antenv.axon_hooks" not in sys.modules:
        m = types.ModuleType("antenv.axon_hooks")
        _hook = [None]

        def set_axon_ntff_profile_hook(h):
            _hook[0] = h

        def get_axon_ntff_profile_hook():
            if _hook[0] is None:
                try:
                    from trn_agent_boot.trn_boot import _ntff_profile_via_ctypes

                    _hook[0] = _ntff_profile_via_ctypes("/opt/axon/libaxon_pjrt.so")
                except Exception:
                    return None
            return _hook[0]

        m.set_axon_ntff_profile_hook = set_axon_ntff_profile_hook
        m.get_axon_ntff_profile_hook = get_axon_ntff_profile_hook
        sys.modules["antenv.axon_hooks"] = m

    import concourse.bass_utils as bu

    if not getattr(bu.upload_artifacts, "_comp_stubbed", False):
        def _no_upload(tmpdir):
            return f"file://{tmpdir}"

        _no_upload._comp_stubbed = True
        bu.upload_artifacts = _no_upload


def kernel(
    audio=None,
    sample_rate=None,
    threshold=None,
    ratio=None,
    attack=None,
    release=None,
    **_unused,
):
    audio_np = np.asarray(audio, dtype=np.float32).reshape(-1)
    n = audio_np.size
    assert n % NCORES == 0, n
    L = n // NCORES

    th = float(np.float32(threshold))
    rt = float(np.float32(ratio))
    at = float(np.float32(attack))
    rl = float(np.float32(release))

    nc = _get_program(L, th, rt, at, rl)

    in_maps = []
    for k in range(NCORES):
        lo = k * L
        if k == 0:
            # grd(1.0) == 0 for any sane threshold < 0, and the scan state
            # initialized from it is exactly the reference's g0 = 0.
            halo = np.full(W, 1.0, dtype=np.float32)
        else:
            halo = audio_np[lo - W : lo]
        in_maps.append(
            {"audio": np.ascontiguousarray(np.concatenate([halo, audio_np[lo : lo + L]]))}
        )

    from concourse.bass_utils import run_bass_kernel_spmd

    trace = bool(int(os.environ.get("COMP_TRACE", "0")))
    if trace:
        _ensure_ntff_hook()
    res = run_bass_kernel_spmd(
        nc, in_maps, core_ids=list(range(NCORES)), trace=trace
    )
    if trace:
        print(f"HW exec time: {res.exec_time_ns} ns")
        print(f"mean exec time: {res.mean_exec_time_ns} ns")
    out = np.concatenate([res.results[k]["out"] for k in range(NCORES)])
    return out
